# revision 13
# baseline (speedup 1.0000x reference)
"""Trainium2 Bass kernel for nn_Conv_39273180955616.

Computes, for X:(16,64,512,512) f32, K:(1,1,7,7), b:(1,1,1,1):
    out[n,c] = correlate2d(X[n,c], Keff, pad=3) + 49*b
where Keff = K.sum(axis=(0,1)).

Pure data parallel over the 1024 (n,c) planes -> 128 planes/core on 8
cores.  The 7x7 correlation runs on TensorE as banded-Toeplitz matmuls
in fp8e4m3 with perf_mode=DoubleRow.  The key trick: each partition
carries one image row as TWO copies shifted by one column, so the two
DoubleRow k-tiles of one 0.5-cycle/row pass accumulate TWO kernel
columns: a 122-row output tile needs only 4 passes (dw pairs 01,23,45,
6-) instead of 7.  The 24-row plane remainders merge 4 planes at a
time into block-diagonal passes.  17 accumulation groups per 4 planes,
2176 matmuls per core.  K is pre-scaled (global scalar minimizing fp8
quantization error) and compensated during PSUM eviction.

All DMA inefficiency lives on the host: inputs are packed
(partition-major, halos, shifted copies and zero pad baked in, fp8) so
each group of 4 planes is ONE ~2.25 MB load with a 17.6 KB contiguous
run per partition; outputs are written bf16 into a packed [122, free]
layout stored with ONE ~2.1 MB SWDGE transfer per group (the host
unpacks/casts to f32).  Loads ride the SP-HWDGE ring, stores the
GpSimd SWDGE ring; PSUM eviction (+bias, scale, bf16 cast) alternates
ScalarE/VectorE.  Matmuls are issued in rounds of 8 PSUM banks with a
shared weight set per 8 consecutive matmuls.
"""
import numpy as np
import ml_dtypes

import concourse.bass as bass
import concourse.tile as tile
from concourse import bacc, mybir
from concourse.bass_utils import run_bass_kernel_spmd

N_CORES = 8
H = 512
W = 512
CPY = 518           # width of each shifted copy (3 + 512 + 3)
SLOT_W = 2 * CPY    # per-partition slot: copy j holds cols j..j+517
GROUP = 4           # planes per group (one load/store per group)
SLOTS = 17          # 16 uniform tiles (4 planes x 4 tiles) + 1 merged bottom
TILE_OH = 122       # valid output rows per uniform tile
BOT_OH = 24         # output rows 488..511, per plane, in the bottom slot
BOT_IN = 30         # input rows per plane in bottom slot (27 real + 3 zero)
N_PASS = 4          # dw pairs (0,1) (2,3) (4,5) (6,zero)
FP8 = ml_dtypes.float8_e4m3
BF16 = ml_dtypes.bfloat16
K_SCALE = 1.68212890625  # minimizes fp8 quantization error of K*s


def _build_weights(Kq: np.ndarray) -> np.ndarray:
    """Kq (7,7) f32 (values on the scaled fp8 grid) -> packed DoubleRow
    lhsT sets [128, 8*256] fp8, layout [k, j*128+m].

    Uniform set p (cols p*256..): w[k, j, m] = Kq[k-m, 2p+j] (0<=k-m<7);
    pass p covers kernel columns 2p and 2p+1 (p=3, j=1 is zero).
    Bottom set p (cols (4+p)*256..): block-diagonal, plane q at
    partitions 30q..30q+29, outputs 24q..24q+23.

    Layout per DoubleRowSwInterleave: flat[k, 2*(127-m)+j] = w[k, j, m]
    (A/B pairs element-interleaved, columns reversed) so the hardware
    weight load reads SBUF contiguously.
    """

    def swi(mat):  # [k, j, m] -> interleaved/reversed flat [k, 256]
        return mat[:, :, ::-1].transpose(0, 2, 1).reshape(128, 256)

    Kx = np.zeros((7, 8), np.float32)
    Kx[:, :7] = Kq
    wt = np.zeros((128, 8 * 256), np.float32)
    k = np.arange(128)[:, None]
    m = np.arange(128)[None, :]
    dh = k - m
    ok = (dh >= 0) & (dh < 7)
    u = np.arange(BOT_IN)[:, None]
    i = np.arange(BOT_OH)[None, :]
    dhb = u - i
    okb = (dhb >= 0) & (dhb < 7)
    for p in range(N_PASS):
        mat = np.zeros((128, 2, 128), np.float32)
        for j in range(2):
            mat[:, j, :][ok] = Kx[dh[ok], 2 * p + j]
        wt[:, p * 256:(p + 1) * 256] = swi(mat)
        mat = np.zeros((128, 2, 128), np.float32)
        for j in range(2):
            blk = np.zeros((BOT_IN, BOT_OH), np.float32)
            blk[okb] = Kx[dhb[okb], 2 * p + j]
            for q in range(GROUP):
                mat[BOT_IN * q:BOT_IN * (q + 1), j,
                    BOT_OH * q:BOT_OH * (q + 1)] = blk
        wt[:, (4 + p) * 256:(5 + p) * 256] = swi(mat)
    return wt.astype(FP8)


_NC_CACHE = {}


def _get_module(n_planes: int):
    if n_planes in _NC_CACHE:
        return _NC_CACHE[n_planes]
    assert n_planes % GROUP == 0
    n_groups = n_planes // GROUP
    nf_in = n_groups * SLOTS * SLOT_W
    nf_out = n_groups * SLOTS * W
    nc = bacc.Bacc("TRN2", target_bir_lowering=False, debug=False,
                   num_devices=N_CORES)
    xin = nc.dram_tensor("xin", [128, nf_in], mybir.dt.float8e4,
                         kind="ExternalInput")
    wt = nc.dram_tensor("wt", [128, 8 * 256], mybir.dt.float8e4,
                        kind="ExternalInput")
    bv = nc.dram_tensor("bv", [128, 1], mybir.dt.float32,
                        kind="ExternalInput")
    out = nc.dram_tensor("out", [TILE_OH, nf_out], mybir.dt.bfloat16,
                         kind="ExternalOutput")
    inv_s = 1.0 / K_SCALE

    with tile.TileContext(nc) as tc:
        with (
            tc.tile_pool(name="wp", bufs=1) as wpool,
            tc.tile_pool(name="xa", bufs=4) as xpoolA,
            tc.tile_pool(name="xb", bufs=4) as xpoolB,
            tc.tile_pool(name="ps", bufs=8, space="PSUM") as pspool,
            tc.tile_pool(name="oa", bufs=3) as opoolA,
            tc.tile_pool(name="obp", bufs=3) as opoolB,
            tc.tile_pool(name="oc", bufs=3) as opoolC,
        ):
            wtile = wpool.tile([128, 8 * 256], mybir.dt.float8e4)
            nc.sync.dma_start(wtile[:], wt.ap())
            btile = wpool.tile([128, 1], mybir.dt.float32)
            nc.sync.dma_start(btile[:], bv.ap())

            def evict(ob, slot_in_buf, s, ps):
                dst = ob[:, slot_in_buf * W:(slot_in_buf + 1) * W]
                if s % 2 == 0:
                    nc.scalar.activation(
                        dst, ps[:, :],
                        mybir.ActivationFunctionType.Identity,
                        bias=btile[:, :], scale=inv_s)
                else:
                    nc.vector.tensor_scalar(
                        dst, ps[:, :], inv_s, btile[:, :],
                        op0=mybir.AluOpType.mult,
                        op1=mybir.AluOpType.add)

            for g in range(n_groups):
                xga = xpoolA.tile([128, 8 * SLOT_W], mybir.dt.float8e4)
                nc.sync.dma_start(
                    xga[:], bass.AP(xin, g * SLOTS * SLOT_W,
                                    [[nf_in, 128], [1, 8 * SLOT_W]]))
                xgb = xpoolB.tile([128, 9 * SLOT_W], mybir.dt.float8e4)
                nc.sync.dma_start(
                    xgb[:], bass.AP(xin, g * SLOTS * SLOT_W + 8 * SLOT_W,
                                    [[nf_in, 128], [1, 9 * SLOT_W]]))
                obA = opoolA.tile([128, 8 * W], mybir.dt.bfloat16)
                obB = opoolB.tile([128, 8 * W], mybir.dt.bfloat16)
                obC = opoolC.tile([128, W], mybir.dt.bfloat16)
                # rounds of 8 slots -> 8 live PSUM banks; 8 consecutive
                # matmuls share one weight set; each chunk of the group's
                # output is stored as soon as its evictions are done
                for r0 in range(0, SLOTS, 8):
                    slots = range(r0, min(r0 + 8, SLOTS))
                    pst = {s: pspool.tile([128, W], mybir.dt.float32,
                                          name="ps")
                           for s in slots}
                    for p in range(N_PASS):
                        for s in slots:
                            wset = (4 + p) if s == SLOTS - 1 else p
                            if s < 8:
                                src_ = xga[:, s * SLOT_W:(s + 1) * SLOT_W]
                            else:
                                src_ = xgb[:, (s - 8) * SLOT_W:
                                           (s - 7) * SLOT_W]
                            rhs = src_.rearrange("p (j w) -> p j w", j=2)[
                                :, :, 2 * p:2 * p + W]
                            nc.tensor.matmul(
                                pst[s][:, :],
                                wtile[:, wset * 256:(wset + 1) * 256
                                      ].rearrange("p (j m) -> p j m", j=2),
                                rhs,
                                start=(p == 0), stop=(p == N_PASS - 1),
                                perf_mode=mybir.MatmulPerfMode.DoubleRowSwInterleave)
                    for s in slots:
                        if s < 8:
                            evict(obA, s, s, pst[s])
                        elif s < 16:
                            evict(obB, s - 8, s, pst[s])
                        else:
                            evict(obC, 0, s, pst[s])
                    if slots[-1] == 7:
                        nc.gpsimd.dma_start(
                            bass.AP(out, g * SLOTS * W,
                                    [[nf_out, TILE_OH], [1, 8 * W]]),
                            obA[:TILE_OH, :])
                    elif slots[-1] == 15:
                        nc.gpsimd.dma_start(
                            bass.AP(out, g * SLOTS * W + 8 * W,
                                    [[nf_out, TILE_OH], [1, 8 * W]]),
                            obB[:TILE_OH, :])
                nc.gpsimd.dma_start(
                    bass.AP(out, g * SLOTS * W + 16 * W,
                            [[nf_out, TILE_OH], [1, W]]),
                    obC[:TILE_OH, :])

    nc.compile()
    _NC_CACHE[n_planes] = nc
    return nc


def _prep_inputs(X, K, b, n_cores=N_CORES):
    Keff = np.asarray(K, np.float32).sum(axis=(0, 1))
    Kq = (Keff * K_SCALE).astype(FP8).astype(np.float32)
    wt = _build_weights(Kq)
    bias = np.float32(np.asarray(b).reshape(-1)[0]) * np.float32(K.size)
    bv = np.full((128, 1), bias, np.float32)

    Xr = np.asarray(X).reshape(-1, H, W)
    n_total = Xr.shape[0]
    per = n_total // n_cores
    n_groups = per // GROUP
    nf_in = n_groups * SLOTS * SLOT_W

    in_maps = []
    for c in range(n_cores):
        # padded planes: pad row = image row + 3 (0..517), col likewise;
        # width 519 so the one-shifted copy still ends in zero pad
        Xpad = np.zeros((per, H + 6, CPY + 1), FP8)
        Xpad[:, 3:3 + H, 3:3 + W] = Xr[c * per:(c + 1) * per]
        P4 = Xpad.reshape(n_groups, GROUP, H + 6, CPY + 1)
        xin = np.zeros((128, n_groups, SLOTS, 2, CPY), FP8)
        xu = xin[:, :, :SLOTS - 1].reshape(128, n_groups, GROUP, 4, 2, CPY)
        for t in range(4):
            # tile t: partition k = image row 122t - 3 + k = pad idx 122t + k
            blk = P4[:, :, 122 * t:122 * t + 128, :]
            for j in range(2):
                xu[:, :, :, t, j, :] = blk[..., j:j + CPY].transpose(
                    2, 0, 1, 3)
        # bottom slot: partition 30q + u = plane q pad idx 488 + u
        blk = P4[:, :, 488:488 + BOT_IN, :]
        for j in range(2):
            xin[:GROUP * BOT_IN, :, SLOTS - 1, j, :] = (
                blk[..., j:j + CPY].transpose(1, 2, 0, 3).reshape(
                    GROUP * BOT_IN, n_groups, CPY))
        in_maps.append({"xin": np.ascontiguousarray(xin.reshape(128, nf_in)),
                        "wt": wt, "bv": bv})
    return in_maps, per


def _unpack_output(res, per, shape):
    n_groups = per // GROUP
    n_cores = len(res.results)
    out = np.empty((n_cores * per, H, W), np.float32)
    O4 = out.reshape(n_cores, n_groups, GROUP, H, W)
    for c in range(n_cores):
        ob = res.results[c]["out"].reshape(TILE_OH, n_groups, SLOTS, W)
        U = ob[:, :, :SLOTS - 1, :].reshape(TILE_OH, n_groups, GROUP, 4, W)
        for t in range(4):
            O4[c, :, :, 122 * t:122 * t + TILE_OH, :] = (
                U[:, :, :, t, :].transpose(1, 2, 0, 3))
        B = ob[:GROUP * BOT_OH, :, SLOTS - 1, :].reshape(
            GROUP, BOT_OH, n_groups, W)
        O4[c, :, :, 488:488 + BOT_OH, :] = B.transpose(2, 0, 1, 3)
    return out.reshape(shape)


def kernel(X, K, b):
    in_maps, per = _prep_inputs(X, K, b)
    nc = _get_module(per)
    res = run_bass_kernel_spmd(nc, in_maps, list(range(N_CORES)))
    return _unpack_output(res, per, np.asarray(X).shape)


# revision 15
# speedup vs baseline: 1.0477x; 1.0477x over previous
"""Trainium2 Bass kernel for nn_Conv_39273180955616.

Computes, for X:(16,64,512,512) f32, K:(1,1,7,7), b:(1,1,1,1):
    out[n,c] = correlate2d(X[n,c], Keff, pad=3) + 49*b
where Keff = K.sum(axis=(0,1)).

Pure data parallel over the 1024 (n,c) planes -> 128 planes/core on 8
cores.  The 7x7 correlation runs on TensorE as banded-Toeplitz matmuls
in fp8e4m3 with perf_mode=DoubleRow.  The key trick: each partition
carries one image row as TWO copies shifted by one column, so the two
DoubleRow k-tiles of one 0.5-cycle/row pass accumulate TWO kernel
columns: a 122-row output tile needs only 4 passes (dw pairs 01,23,45,
6-) instead of 7.  The 24-row plane remainders merge 4 planes at a
time into block-diagonal passes.  17 accumulation groups per 4 planes,
2176 matmuls per core.  K is pre-scaled (global scalar minimizing fp8
quantization error) and compensated during PSUM eviction.

All DMA inefficiency lives on the host: inputs are packed
(partition-major, halos, shifted copies and zero pad baked in, fp8) so
each group of 4 planes is ONE ~2.25 MB load with a 17.6 KB contiguous
run per partition; outputs are written bf16 into a packed [122, free]
layout stored with ONE ~2.1 MB SWDGE transfer per group (the host
unpacks/casts to f32).  Loads ride the SP-HWDGE ring, stores the
GpSimd SWDGE ring; PSUM eviction (+bias, scale, bf16 cast) alternates
ScalarE/VectorE.  Matmuls are issued in rounds of 8 PSUM banks with a
shared weight set per 8 consecutive matmuls.
"""
import numpy as np
import ml_dtypes

import concourse.bass as bass
import concourse.tile as tile
from concourse import bacc, mybir
from concourse.bass_utils import run_bass_kernel_spmd

N_CORES = 8
H = 512
W = 512
CPY = 518           # width of each shifted copy (3 + 512 + 3)
SLOT_W = 2 * CPY    # per-partition slot: copy j holds cols j..j+517
GROUP = 4           # planes per group (one load/store per group)
SLOTS = 17          # 16 uniform tiles (4 planes x 4 tiles) + 1 merged bottom
TILE_OH = 122       # valid output rows per uniform tile
BOT_OH = 24         # output rows 488..511, per plane, in the bottom slot
BOT_IN = 30         # input rows per plane in bottom slot (27 real + 3 zero)
N_PASS = 4          # dw pairs (0,1) (2,3) (4,5) (6,zero)
FP8 = ml_dtypes.float8_e4m3
BF16 = ml_dtypes.bfloat16
K_SCALE = 1.68212890625  # minimizes fp8 quantization error of K*s


def _build_weights(Kq: np.ndarray) -> np.ndarray:
    """Kq (7,7) f32 (values on the scaled fp8 grid) -> packed DoubleRow
    lhsT sets [128, 8*256] fp8, layout [k, j*128+m].

    Uniform set p (cols p*256..): w[k, j, m] = Kq[k-m, 2p+j] (0<=k-m<7);
    pass p covers kernel columns 2p and 2p+1 (p=3, j=1 is zero).
    Bottom set p (cols (4+p)*256..): block-diagonal, plane q at
    partitions 30q..30q+29, outputs 24q..24q+23.

    Layout per DoubleRowSwInterleave: flat[k, 2*(127-m)+j] = w[k, j, m]
    (A/B pairs element-interleaved, columns reversed) so the hardware
    weight load reads SBUF contiguously.
    """

    def swi(mat):  # [k, j, m] -> interleaved/reversed flat [k, 256]
        return mat[:, :, ::-1].transpose(0, 2, 1).reshape(128, 256)

    Kx = np.zeros((7, 8), np.float32)
    Kx[:, :7] = Kq
    wt = np.zeros((128, 8 * 256), np.float32)
    k = np.arange(128)[:, None]
    m = np.arange(128)[None, :]
    dh = k - m
    ok = (dh >= 0) & (dh < 7)
    u = np.arange(BOT_IN)[:, None]
    i = np.arange(BOT_OH)[None, :]
    dhb = u - i
    okb = (dhb >= 0) & (dhb < 7)
    for p in range(N_PASS):
        mat = np.zeros((128, 2, 128), np.float32)
        for j in range(2):
            mat[:, j, :][ok] = Kx[dh[ok], 2 * p + j]
        wt[:, p * 256:(p + 1) * 256] = swi(mat)
        mat = np.zeros((128, 2, 128), np.float32)
        for j in range(2):
            blk = np.zeros((BOT_IN, BOT_OH), np.float32)
            blk[okb] = Kx[dhb[okb], 2 * p + j]
            for q in range(GROUP):
                mat[BOT_IN * q:BOT_IN * (q + 1), j,
                    BOT_OH * q:BOT_OH * (q + 1)] = blk
        wt[:, (4 + p) * 256:(5 + p) * 256] = swi(mat)
    return wt.astype(FP8)


_NC_CACHE = {}


def _get_module(n_planes: int):
    if n_planes in _NC_CACHE:
        return _NC_CACHE[n_planes]
    assert n_planes % GROUP == 0
    n_groups = n_planes // GROUP
    nf_in = n_groups * SLOTS * SLOT_W
    nf_out = n_groups * SLOTS * W
    nc = bacc.Bacc("TRN2", target_bir_lowering=False, debug=False,
                   num_devices=N_CORES)
    xin = nc.dram_tensor("xin", [128, nf_in], mybir.dt.float8e4,
                         kind="ExternalInput")
    wt = nc.dram_tensor("wt", [128, 8 * 256], mybir.dt.float8e4,
                        kind="ExternalInput")
    bv = nc.dram_tensor("bv", [128, 1], mybir.dt.float32,
                        kind="ExternalInput")
    out = nc.dram_tensor("out", [TILE_OH, nf_out], mybir.dt.bfloat16,
                         kind="ExternalOutput")
    inv_s = 1.0 / K_SCALE

    with tile.TileContext(nc) as tc:
        with (
            tc.tile_pool(name="wp", bufs=1) as wpool,
            tc.tile_pool(name="xa", bufs=1) as xpoolA,
            tc.tile_pool(name="xb", bufs=4) as xpoolB,
            tc.tile_pool(name="ps", bufs=8, space="PSUM") as pspool,
            tc.tile_pool(name="oa", bufs=3) as opoolA,
            tc.tile_pool(name="obp", bufs=3) as opoolB,
        ):
            wtile = wpool.tile([128, 8 * 256], mybir.dt.float8e4)
            nc.sync.dma_start(wtile[:], wt.ap())
            btile = wpool.tile([128, 1], mybir.dt.float32)
            nc.sync.dma_start(btile[:], bv.ap())

            def evict(ob, slot_in_buf, s, ps):
                dst = ob[:, slot_in_buf * W:(slot_in_buf + 1) * W]
                if s % 2 == 0:
                    nc.scalar.activation(
                        dst, ps[:, :],
                        mybir.ActivationFunctionType.Identity,
                        bias=btile[:, :], scale=inv_s)
                else:
                    nc.vector.tensor_scalar(
                        dst, ps[:, :], inv_s, btile[:, :],
                        op0=mybir.AluOpType.mult,
                        op1=mybir.AluOpType.add)

            for g in range(n_groups):
                first, last = g == 0, g == n_groups - 1
                if first:
                    # split the first load so compute starts sooner
                    xga = xpoolA.tile([128, 8 * SLOT_W],
                                      mybir.dt.float8e4, bufs=1)
                    nc.sync.dma_start(
                        xga[:], bass.AP(xin, 0,
                                        [[nf_in, 128], [1, 8 * SLOT_W]]))
                    xgb = xpoolB.tile([128, 9 * SLOT_W],
                                      mybir.dt.float8e4, bufs=1)
                    nc.sync.dma_start(
                        xgb[:], bass.AP(xin, 8 * SLOT_W,
                                        [[nf_in, 128], [1, 9 * SLOT_W]]))
                else:
                    xgb = xpoolB.tile([128, SLOTS * SLOT_W],
                                      mybir.dt.float8e4, name="xgf")
                    nc.sync.dma_start(
                        xgb[:], bass.AP(xin, g * SLOTS * SLOT_W,
                                        [[nf_in, 128],
                                         [1, SLOTS * SLOT_W]]))
                    xga = xgb
                obA = opoolA.tile([128, 8 * W], mybir.dt.bfloat16)
                obB = opoolB.tile([128, 9 * W], mybir.dt.bfloat16)
                # rounds of 8 slots -> 8 live PSUM banks; 8 consecutive
                # matmuls share one weight set; each half of the group's
                # output is stored as soon as its evictions are done
                for r0 in range(0, SLOTS, 8):
                    slots = range(r0, min(r0 + 8, SLOTS))
                    pst = {s: pspool.tile([128, W], mybir.dt.float32,
                                          name="ps")
                           for s in slots}
                    for p in range(N_PASS):
                        for s in slots:
                            wset = (4 + p) if s == SLOTS - 1 else p
                            if first and s < 8:
                                src_ = xga[:, s * SLOT_W:(s + 1) * SLOT_W]
                            elif first:
                                src_ = xgb[:, (s - 8) * SLOT_W:
                                           (s - 7) * SLOT_W]
                            else:
                                src_ = xgb[:, s * SLOT_W:(s + 1) * SLOT_W]
                            rhs = src_.rearrange("p (j w) -> p j w", j=2)[
                                :, :, 2 * p:2 * p + W]
                            nc.tensor.matmul(
                                pst[s][:, :],
                                wtile[:, wset * 256:(wset + 1) * 256
                                      ].rearrange("p (j m) -> p j m", j=2),
                                rhs,
                                start=(p == 0), stop=(p == N_PASS - 1),
                                perf_mode=mybir.MatmulPerfMode.DoubleRowSwInterleave)
                    for s in slots:
                        if s < 8:
                            evict(obA, s, s, pst[s])
                        else:
                            evict(obB, s - 8, s, pst[s])
                    if slots[-1] == 7:
                        nc.gpsimd.dma_start(
                            bass.AP(out, g * SLOTS * W,
                                    [[nf_out, TILE_OH], [1, 8 * W]]),
                            obA[:TILE_OH, :])
                    elif last and slots[-1] == 15:
                        # last group: drain the second half early so the
                        # final store is only the bottom slot
                        nc.gpsimd.dma_start(
                            bass.AP(out, g * SLOTS * W + 8 * W,
                                    [[nf_out, TILE_OH], [1, 8 * W]]),
                            obB[:TILE_OH, :8 * W])
                if last:
                    nc.gpsimd.dma_start(
                        bass.AP(out, g * SLOTS * W + 16 * W,
                                [[nf_out, TILE_OH], [1, W]]),
                        obB[:TILE_OH, 8 * W:9 * W])
                else:
                    nc.gpsimd.dma_start(
                        bass.AP(out, g * SLOTS * W + 8 * W,
                                [[nf_out, TILE_OH], [1, 9 * W]]),
                        obB[:TILE_OH, :])

    nc.compile()
    _NC_CACHE[n_planes] = nc
    return nc


def _prep_inputs(X, K, b, n_cores=N_CORES):
    Keff = np.asarray(K, np.float32).sum(axis=(0, 1))
    Kq = (Keff * K_SCALE).astype(FP8).astype(np.float32)
    wt = _build_weights(Kq)
    bias = np.float32(np.asarray(b).reshape(-1)[0]) * np.float32(K.size)
    bv = np.full((128, 1), bias, np.float32)

    Xr = np.asarray(X).reshape(-1, H, W)
    n_total = Xr.shape[0]
    per = n_total // n_cores
    n_groups = per // GROUP
    nf_in = n_groups * SLOTS * SLOT_W

    in_maps = []
    for c in range(n_cores):
        # padded planes: pad row = image row + 3 (0..517), col likewise;
        # width 519 so the one-shifted copy still ends in zero pad
        Xpad = np.zeros((per, H + 6, CPY + 1), FP8)
        Xpad[:, 3:3 + H, 3:3 + W] = Xr[c * per:(c + 1) * per]
        P4 = Xpad.reshape(n_groups, GROUP, H + 6, CPY + 1)
        xin = np.zeros((128, n_groups, SLOTS, 2, CPY), FP8)
        xu = xin[:, :, :SLOTS - 1].reshape(128, n_groups, GROUP, 4, 2, CPY)
        for t in range(4):
            # tile t: partition k = image row 122t - 3 + k = pad idx 122t + k
            blk = P4[:, :, 122 * t:122 * t + 128, :]
            for j in range(2):
                xu[:, :, :, t, j, :] = blk[..., j:j + CPY].transpose(
                    2, 0, 1, 3)
        # bottom slot: partition 30q + u = plane q pad idx 488 + u
        blk = P4[:, :, 488:488 + BOT_IN, :]
        for j in range(2):
            xin[:GROUP * BOT_IN, :, SLOTS - 1, j, :] = (
                blk[..., j:j + CPY].transpose(1, 2, 0, 3).reshape(
                    GROUP * BOT_IN, n_groups, CPY))
        in_maps.append({"xin": np.ascontiguousarray(xin.reshape(128, nf_in)),
                        "wt": wt, "bv": bv})
    return in_maps, per


def _unpack_output(res, per, shape):
    n_groups = per // GROUP
    n_cores = len(res.results)
    out = np.empty((n_cores * per, H, W), np.float32)
    O4 = out.reshape(n_cores, n_groups, GROUP, H, W)
    for c in range(n_cores):
        ob = res.results[c]["out"].reshape(TILE_OH, n_groups, SLOTS, W)
        U = ob[:, :, :SLOTS - 1, :].reshape(TILE_OH, n_groups, GROUP, 4, W)
        for t in range(4):
            O4[c, :, :, 122 * t:122 * t + TILE_OH, :] = (
                U[:, :, :, t, :].transpose(1, 2, 0, 3))
        B = ob[:GROUP * BOT_OH, :, SLOTS - 1, :].reshape(
            GROUP, BOT_OH, n_groups, W)
        O4[c, :, :, 488:488 + BOT_OH, :] = B.transpose(2, 0, 1, 3)
    return out.reshape(shape)


def kernel(X, K, b):
    in_maps, per = _prep_inputs(X, K, b)
    nc = _get_module(per)
    res = run_bass_kernel_spmd(nc, in_maps, list(range(N_CORES)))
    return _unpack_output(res, per, np.asarray(X).shape)
